# revision 22
# baseline (speedup 1.0000x reference)
"""Trainium2 Bass kernel for the DataReloadingQNN problem.

Math: layers 0..4 of the circuit are sample-independent -> one shared state
v4. Layer 5 applies, per qubit q, shared B_q = RZ RY RZ then the data gate
RY(x_q) = cos(x_q/2) I + sin(x_q/2) J. Expanding the product over qubits
2..10 only:
    state_b = G0(b) G1(b) P sum_m W[b, m] u_m,   m in [0, 512)
where W = tensor product of the per-qubit [cos, sin] pairs for qubits 2..10
(bit q-2 of m picks cos/sin), u_m are 512 shared vectors derived from params
(CNOT permutation P folded in), and G1/G0 are the remaining data gates for
qubits 1 and 0, P-conjugated. Conjugation by P (prefix-parity bit map) turns
qubit 1 into "partner = reversal within each half, sign -++- by quarter" and
qubit 0 into "partner = global reversal, sign -+ by half" -- cheap local
vector ops with reversed access patterns (stride -1 keeps the DVE fast path).

Device work per core (1024 samples, 8 sample-tiles of 128):
  1. cos/sin/-sin of x/2 on ScalarE (post-gate coefficients)
  2. S = W @ U as bf16 matmuls, K=512 (W^T precomputed on host, DMA'd in)
  3. PSUM -> SBUF bf16 drains (2-bank PSUM tiles) split ScalarE/GpSimd
  4. gate q=1 (DVE), gate q=0 (t2 on ScalarE, fused mul-add on DVE -> f32)
  5. DMA out f32 per sample-tile
Inputs are sharded batch-wise across 8 cores; U (params-derived) replicated.
"""
import numpy as np
import ml_dtypes

import concourse.bass as bass
import concourse.bacc as bacc
import concourse.tile as tile
from concourse import mybir
from concourse.bass_utils import run_bass_kernel_spmd

N = 11
DIM = 2048
BATCH = 8192
NCORES = 8
BSH = BATCH // NCORES          # 1024 samples per core
NTILES = BSH // 128            # 8 sample-tiles per core
KDIM = 512                     # 2**9 expanded patterns (qubits 2..10)
KT = KDIM // 128               # 4 k-tiles
NPS = 4                        # PSUM tiles per sample-tile (2 banks each)
PSW = (2 * DIM) // NPS         # 1024 f32 columns per PSUM tile
NMM = PSW // 512               # matmul column splits per PSUM tile
F32 = mybir.dt.float32
BF16 = mybir.dt.bfloat16

# ---------------------------------------------------------------- host math

def _rz(phi):
    e = np.exp(-0.5j * phi)
    return np.array([[e, 0], [0, np.conj(e)]], dtype=np.complex128)


def _ry(theta):
    t = 0.5 * theta
    c, s = np.cos(t), np.sin(t)
    return np.array([[c, -s], [s, c]], dtype=np.complex128)


def _apply_1q_rows(rows, U, q):
    R = rows.shape[0]
    st = rows.reshape(R, 2 ** q, 2, 2 ** (N - 1 - q))
    st = np.einsum('ab,rxby->rxay', U, st)
    return st.reshape(R, DIM)


def _apply_cnot_rows(rows, c):
    R = rows.shape[0]
    st = rows.reshape(R, 2 ** c, 2, 2, 2 ** (N - 2 - c))
    st = np.stack([st[:, :, 0], st[:, :, 1, ::-1]], axis=2)
    return st.reshape(R, DIM)


def build_u_matrix(params):
    """(6,11,3) f32 -> U (512, 4096) f64, re/im interleaved columns.
    Data-gate expansion over qubits 2..10 (qubit q -> bit q-2); B_0, B_1
    folded in unexpanded (their data gates run on-chip post-matmul)."""
    p = params.astype(np.float64)
    v = np.zeros((1, DIM), dtype=np.complex128)
    v[0, 0] = 1.0
    for l in range(5):
        for q in range(N):
            v = _apply_1q_rows(v, _rz(p[l, q, 0]), q)
            v = _apply_1q_rows(v, _ry(p[l, q, 1]), q)
            v = _apply_1q_rows(v, _rz(p[l, q, 2]), q)
        for c in range(N - 1):
            v = _apply_cnot_rows(v, c)

    J = np.array([[0, -1], [1, 0]], dtype=np.complex128)
    rows = v
    for q in (0, 1):
        Bq = _rz(p[5, q, 2]) @ _ry(p[5, q, 1]) @ _rz(p[5, q, 0])
        rows = _apply_1q_rows(rows, Bq, q)
    for q in range(2, N):
        Bq = _rz(p[5, q, 2]) @ _ry(p[5, q, 1]) @ _rz(p[5, q, 0])
        rb = _apply_1q_rows(rows, Bq, q)
        rc = _apply_1q_rows(rows, J @ Bq, q)
        rows = np.concatenate([rb, rc], axis=0)

    g = np.arange(DIM)[None, :]
    for c in range(N - 1):
        g = _apply_cnot_rows(g.astype(np.float64), c).astype(np.int64)
    rows = rows[:, g[0]]

    U = np.empty((KDIM, 2 * DIM), dtype=np.float64)
    U[:, 0::2] = rows.real
    U[:, 1::2] = rows.imag
    return U


def build_wt(X):
    """Per-sample Kronecker coefficients over qubits 2..10, transposed for
    the PE: returns (KT, 128, BATCH) f32 with wt[k, kk, b] = W[b, 128k+kk].
    (Sliced per core by the driver.)"""
    c = np.cos(X * 0.5)
    s = np.sin(X * 0.5)
    B = X.shape[0]
    W = np.ones((B, 1), dtype=np.float32)
    for q in range(2, N):
        W = np.concatenate([W * c[:, q:q + 1], W * s[:, q:q + 1]], axis=1)
    return np.ascontiguousarray(W.T.reshape(KT, 128, B))

# ------------------------------------------------------------- bass kernel

def build_kernel():
    nc = bacc.Bacc()
    x_d = nc.dram_tensor("x", (BSH, N), F32, kind="ExternalInput")
    wt_d = nc.dram_tensor("wt", (KT, 128, BSH), BF16, kind="ExternalInput")
    u_d = nc.dram_tensor("u", (KT, 128, 2 * DIM), BF16, kind="ExternalInput")
    out_d = nc.dram_tensor("out", (BSH, 2 * DIM), F32, kind="ExternalOutput")

    MUL = mybir.AluOpType.mult
    ADD = mybir.AluOpType.add

    with tile.TileContext(nc) as tc:
        with (
            tc.tile_pool(name="const", bufs=1) as const_pool,
            tc.tile_pool(name="st", bufs=2) as st_pool,
            tc.tile_pool(name="outs", bufs=2) as out_pool,
            tc.tile_pool(name="pmm", bufs=4, space=bass.MemorySpace.PSUM) as pmm_pool,
        ):
            # x: (1024, 11) -> sbuf (128, 8*11); sample-tile t in cols
            # [t*11, (t+1)*11)
            x_sb = const_pool.tile([128, NTILES * N], F32)
            x_r = x_d.rearrange("(t p) f -> p t f", p=128)
            nc.gpsimd.dma_start(x_sb[:].rearrange("p (t f) -> p t f", f=N), x_r)

            cos_sb = const_pool.tile([128, NTILES * N], F32)
            sin_sb = const_pool.tile([128, NTILES * N], F32)
            nsin_sb = const_pool.tile([128, NTILES * N], F32)
            hp_t = const_pool.tile([128, 1], F32)
            zr_t = const_pool.tile([128, 1], F32)
            nc.vector.memset(hp_t[:], float(np.pi / 2))
            nc.vector.memset(zr_t[:], 0.0)
            # cos(t) = sin(pi/2 - t): keeps Sin args in (-pi/2, pi/2], the
            # ACT table is inaccurate beyond pi
            nc.scalar.activation(cos_sb[:], x_sb[:],
                                 mybir.ActivationFunctionType.Sin,
                                 bias=hp_t[:], scale=-0.5)
            nc.scalar.activation(sin_sb[:], x_sb[:],
                                 mybir.ActivationFunctionType.Sin,
                                 bias=zr_t[:], scale=0.5)
            nc.scalar.activation(nsin_sb[:], x_sb[:],
                                 mybir.ActivationFunctionType.Sin,
                                 bias=zr_t[:], scale=-0.5)

            # W^T: 4 k-tiles of (128, 1024) bf16; U: 4 k-tiles of (128, 4096)
            # interleaved k-order on two trigger queues so matmul k can start
            # as soon as its (wt[k], u[k]) pair lands
            wt_sb, u_sb = [], []
            for k in range(KT):
                wtt = const_pool.tile([128, BSH], BF16, tag=f"w{k}")
                wt_sb.append(wtt)
                ut = const_pool.tile([128, 2 * DIM], BF16, tag=f"u{k}")
                u_sb.append(ut)
            for k in range(KT):
                nc.sync.dma_start(wt_sb[k][:], wt_d[k])
                nc.gpsimd.dma_start(u_sb[k][:], u_d[k])

            a_tiles = {}

            def emit_mm_drain(t):
                # ---- matmul K=512, output 4096 f32 in 4 PSUM tiles
                a = st_pool.tile([128, 2 * DIM], BF16, tag="a")
                a_tiles[t] = a
                for pi in range(NPS):
                    pmm = pmm_pool.tile([128, PSW], F32)
                    # k outer: consecutive matmuls share stationary weights
                    for k in range(KT):
                        for mi in range(NMM):
                            cl = pi * PSW + mi * 512
                            nc.tensor.matmul(
                                pmm[:, mi * 512:(mi + 1) * 512],
                                wt_sb[k][:, t * 128:(t + 1) * 128],
                                u_sb[k][:, cl:cl + 512],
                                start=(k == 0), stop=(k == KT - 1))
                    # drain (f32 -> bf16) on ScalarE (GpSimd can't read PSUM)
                    nc.scalar.copy(a[:, pi * PSW:(pi + 1) * PSW], pmm[:])

            def emit_gates(t):
                col = t * N
                c1 = cos_sb[:, col + 1:col + 2]
                s1p = sin_sb[:, col + 1:col + 2]
                s1n = nsin_sb[:, col + 1:col + 2]
                c0 = cos_sb[:, col + 0:col + 1]
                s0p = sin_sb[:, col + 0:col + 1]
                s0n = nsin_sb[:, col + 0:col + 1]
                a = a_tiles.pop(t)
                # ---- gate q=1: quarters of 512 complex, partner = reversal
                # within the other quarter of the same half, sign -++-.
                # Both muls contiguous (4x); sign + partner handled by the
                # quarter add/subs whose in1 reads reversed (2x).
                t1 = st_pool.tile([128, 2 * DIM], BF16, tag="t1")
                nc.vector.tensor_scalar_mul(t1[:], a[:], c1)
                ts = st_pool.tile([128, 2 * DIM], BF16, tag="ts")
                nc.vector.tensor_scalar_mul(ts[:], a[:], s1p)
                tsv = ts[:].rearrange("p (c r) -> p c r", r=2)
                t1v = t1[:].rearrange("p (c r) -> p c r", r=2)
                g1 = st_pool.tile([128, 2 * DIM], BF16, tag="g1")
                gv = g1[:].rearrange("p (c r) -> p c r", r=2)
                Q = 512
                SUB = mybir.AluOpType.subtract
                quarter_src = [(1, SUB), (0, ADD), (3, ADD), (2, SUB)]
                for qi, (pq, op) in enumerate(quarter_src):
                    nc.vector.tensor_tensor(
                        gv[:, qi * Q:(qi + 1) * Q, :],
                        t1v[:, qi * Q:(qi + 1) * Q, :],
                        tsv[:, pq * Q:(pq + 1) * Q, :][:, ::-1, :], op)

                # ---- gate q=0: halves of 1024 complex, partner = global
                # reversal, sign -+; t2 on ScalarE (DVE for the last tile:
                # shortens the pipeline tail), fused mul-add on DVE
                t2 = st_pool.tile([128, 2 * DIM], BF16, tag="t2")
                if t == NTILES - 1:
                    nc.vector.tensor_scalar_mul(t2[:], g1[:], c0)
                else:
                    nc.scalar.mul(t2[:], g1[:], c0)
                ot = out_pool.tile([128, 2 * DIM], F32, tag="o")
                ov = ot[:].rearrange("p (c r) -> p c r", r=2)
                t2v = t2[:].rearrange("p (c r) -> p c r", r=2)
                H = 1024
                nc.vector.scalar_tensor_tensor(
                    ov[:, 0:H, :],
                    gv[:, H:2 * H, :][:, ::-1, :], s0n, t2v[:, 0:H, :],
                    MUL, ADD)
                nc.sync.dma_start(out_d[t * 128:(t + 1) * 128, 0:2 * H],
                                  ot[:, 0:2 * H])
                nc.vector.scalar_tensor_tensor(
                    ov[:, H:2 * H, :],
                    gv[:, 0:H, :][:, ::-1, :], s0p, t2v[:, H:2 * H, :],
                    MUL, ADD)
                nc.sync.dma_start(out_d[t * 128:(t + 1) * 128, 2 * H:],
                                  ot[:, 2 * H:])

            # software pipeline: gates for tile t-1 emitted after tile t's
            # matmuls+drains so no engine stalls at the head of its queue
            for t in range(NTILES):
                emit_mm_drain(t)
                if t >= 1:
                    emit_gates(t - 1)
            emit_gates(NTILES - 1)
    nc.finalize()
    return nc

# ----------------------------------------------------------------- driver

_CACHE = {}


def make_in_maps(X, params):
    X = np.ascontiguousarray(np.asarray(X, dtype=np.float32))
    params = np.asarray(params, dtype=np.float32)
    U = build_u_matrix(params)
    u_bf = np.ascontiguousarray(
        U.reshape(KT, 128, 2 * DIM).astype(ml_dtypes.bfloat16))
    WT = build_wt(X)
    in_maps = []
    for c in range(NCORES):
        in_maps.append({
            "x": X[c * BSH:(c + 1) * BSH],
            "wt": np.ascontiguousarray(
                WT[:, :, c * BSH:(c + 1) * BSH].astype(ml_dtypes.bfloat16)),
            "u": u_bf,
        })
    return in_maps


def kernel(X, params):
    if "nc" not in _CACHE:
        _CACHE["nc"] = build_kernel()
    nc = _CACHE["nc"]
    in_maps = make_in_maps(X, params)
    res = run_bass_kernel_spmd(nc, in_maps, list(range(NCORES)))
    out = np.concatenate([res.results[c]["out"] for c in range(NCORES)], axis=0)
    return out.reshape(BATCH, DIM, 2)


# revision 24
# speedup vs baseline: 1.1874x; 1.1874x over previous
"""Trainium2 Bass kernel for the DataReloadingQNN problem.

Math: layers 0..4 of the circuit are sample-independent -> one shared state
v4. Layer 5 applies, per qubit q, shared B_q = RZ RY RZ then the data gate
RY(x_q) = cos(x_q/2) I + sin(x_q/2) J. Expanding the product over qubits
2..10 only:
    state_b = G0(b) G1(b) P sum_m W[b, m] u_m,   m in [0, 512)
where W = tensor product of the per-qubit [cos, sin] pairs for qubits 2..10
(bit q-2 of m picks cos/sin), u_m are 512 shared vectors derived from params
(CNOT permutation P folded in), and G1/G0 are the remaining data gates for
qubits 1 and 0, P-conjugated. Conjugation by P (prefix-parity bit map) turns
qubit 1 into "partner = reversal within each half, sign -++- by quarter" and
qubit 0 into "partner = global reversal, sign -+ by half" -- cheap local
vector ops with reversed access patterns (stride -1 keeps the DVE fast path).

Device work per core (1024 samples, 8 sample-tiles of 128):
  1. cos/sin/-sin of x/2 on ScalarE (post-gate coefficients)
  2. S = W @ U as bf16 matmuls, K=512 (W^T precomputed on host, DMA'd in)
  3. PSUM -> SBUF bf16 drains (2-bank PSUM tiles) split ScalarE/GpSimd
  4. gate q=1 (DVE), gate q=0 (t2 on ScalarE, fused mul-add on DVE -> f32)
  5. DMA out f32 per sample-tile
Inputs are sharded batch-wise across 8 cores; U (params-derived) replicated.
"""
import numpy as np
import ml_dtypes

import concourse.bass as bass
import concourse.bacc as bacc
import concourse.tile as tile
from concourse import mybir
from concourse.bass_utils import run_bass_kernel_spmd

N = 11
DIM = 2048
BATCH = 8192
NCORES = 8
BSH = BATCH // NCORES          # 1024 samples per core
NTILES = BSH // 128            # 8 sample-tiles per core
KDIM = 512                     # 2**9 expanded patterns (qubits 2..10)
KT = KDIM // 128               # 4 k-tiles
NPS = 4                        # PSUM tiles per sample-tile (2 banks each)
PSW = (2 * DIM) // NPS         # 1024 f32 columns per PSUM tile
NMM = PSW // 512               # matmul column splits per PSUM tile
F32 = mybir.dt.float32
BF16 = mybir.dt.bfloat16

# ---------------------------------------------------------------- host math

def _rz(phi):
    e = np.exp(-0.5j * phi)
    return np.array([[e, 0], [0, np.conj(e)]], dtype=np.complex128)


def _ry(theta):
    t = 0.5 * theta
    c, s = np.cos(t), np.sin(t)
    return np.array([[c, -s], [s, c]], dtype=np.complex128)


def _apply_1q_rows(rows, U, q):
    R = rows.shape[0]
    st = rows.reshape(R, 2 ** q, 2, 2 ** (N - 1 - q))
    st = np.einsum('ab,rxby->rxay', U, st)
    return st.reshape(R, DIM)


def _apply_cnot_rows(rows, c):
    R = rows.shape[0]
    st = rows.reshape(R, 2 ** c, 2, 2, 2 ** (N - 2 - c))
    st = np.stack([st[:, :, 0], st[:, :, 1, ::-1]], axis=2)
    return st.reshape(R, DIM)


def build_u_matrix(params):
    """(6,11,3) f32 -> U (512, 4096) f64, re/im interleaved columns.
    Data-gate expansion over qubits 2..10 (qubit q -> bit q-2); B_0, B_1
    folded in unexpanded (their data gates run on-chip post-matmul)."""
    p = params.astype(np.float64)
    v = np.zeros((1, DIM), dtype=np.complex128)
    v[0, 0] = 1.0
    for l in range(5):
        for q in range(N):
            v = _apply_1q_rows(v, _rz(p[l, q, 0]), q)
            v = _apply_1q_rows(v, _ry(p[l, q, 1]), q)
            v = _apply_1q_rows(v, _rz(p[l, q, 2]), q)
        for c in range(N - 1):
            v = _apply_cnot_rows(v, c)

    J = np.array([[0, -1], [1, 0]], dtype=np.complex128)
    rows = v
    for q in (0, 1):
        Bq = _rz(p[5, q, 2]) @ _ry(p[5, q, 1]) @ _rz(p[5, q, 0])
        rows = _apply_1q_rows(rows, Bq, q)
    for q in range(2, N):
        Bq = _rz(p[5, q, 2]) @ _ry(p[5, q, 1]) @ _rz(p[5, q, 0])
        rb = _apply_1q_rows(rows, Bq, q)
        rc = _apply_1q_rows(rows, J @ Bq, q)
        rows = np.concatenate([rb, rc], axis=0)

    g = np.arange(DIM)[None, :]
    for c in range(N - 1):
        g = _apply_cnot_rows(g.astype(np.float64), c).astype(np.int64)
    rows = rows[:, g[0]]

    U = np.empty((KDIM, 2 * DIM), dtype=np.float64)
    U[:, 0::2] = rows.real
    U[:, 1::2] = rows.imag
    return U


def build_wt(X):
    """Per-sample Kronecker coefficients over qubits 2..10, transposed for
    the PE: returns (KT, 128, BATCH) f32 with wt[k, kk, b] = W[b, 128k+kk].
    (Sliced per core by the driver.)"""
    c = np.cos(X * 0.5)
    s = np.sin(X * 0.5)
    B = X.shape[0]
    W = np.ones((B, 1), dtype=np.float32)
    for q in range(2, N):
        W = np.concatenate([W * c[:, q:q + 1], W * s[:, q:q + 1]], axis=1)
    return np.ascontiguousarray(W.T.reshape(KT, 128, B))

# ------------------------------------------------------------- bass kernel

def build_kernel():
    nc = bacc.Bacc()
    x_d = nc.dram_tensor("x", (BSH, N), F32, kind="ExternalInput")
    wt_d = nc.dram_tensor("wt", (KT, 128, BSH), BF16, kind="ExternalInput")
    u_d = nc.dram_tensor("u", (KT, 128, 2 * DIM), BF16, kind="ExternalInput")
    out_d = nc.dram_tensor("out", (BSH, 2 * DIM), F32, kind="ExternalOutput")

    MUL = mybir.AluOpType.mult
    ADD = mybir.AluOpType.add

    with tile.TileContext(nc) as tc:
        with (
            tc.tile_pool(name="const", bufs=1) as const_pool,
            tc.tile_pool(name="st", bufs=2) as st_pool,
            tc.tile_pool(name="outs", bufs=2) as out_pool,
            tc.tile_pool(name="pmm", bufs=4, space=bass.MemorySpace.PSUM) as pmm_pool,
        ):
            # x: (1024, 11) -> sbuf (128, 8*11); sample-tile t in cols
            # [t*11, (t+1)*11)
            x_sb = const_pool.tile([128, NTILES * N], F32)
            x_r = x_d.rearrange("(t p) f -> p t f", p=128)
            nc.gpsimd.dma_start(x_sb[:].rearrange("p (t f) -> p t f", f=N), x_r)

            cos_sb = const_pool.tile([128, NTILES * N], F32)
            sin_sb = const_pool.tile([128, NTILES * N], F32)
            nsin_sb = const_pool.tile([128, NTILES * N], F32)
            hp_t = const_pool.tile([128, 1], F32)
            zr_t = const_pool.tile([128, 1], F32)
            nc.vector.memset(hp_t[:], float(np.pi / 2))
            nc.vector.memset(zr_t[:], 0.0)
            # cos(t) = sin(pi/2 - t): keeps Sin args in (-pi/2, pi/2], the
            # ACT table is inaccurate beyond pi
            nc.scalar.activation(cos_sb[:], x_sb[:],
                                 mybir.ActivationFunctionType.Sin,
                                 bias=hp_t[:], scale=-0.5)
            nc.scalar.activation(sin_sb[:], x_sb[:],
                                 mybir.ActivationFunctionType.Sin,
                                 bias=zr_t[:], scale=0.5)
            nc.scalar.activation(nsin_sb[:], x_sb[:],
                                 mybir.ActivationFunctionType.Sin,
                                 bias=zr_t[:], scale=-0.5)

            # W^T: 4 k-tiles of (128, 1024) bf16; U: 4 k-tiles of (128, 4096)
            # interleaved k-order on two trigger queues so matmul k can start
            # as soon as its (wt[k], u[k]) pair lands
            wt_sb, u_sb = [], []
            for k in range(KT):
                wtt = const_pool.tile([128, BSH], BF16, tag=f"w{k}")
                wt_sb.append(wtt)
                ut = const_pool.tile([128, 2 * DIM], BF16, tag=f"u{k}")
                u_sb.append(ut)
            for k in range(KT):
                nc.sync.dma_start(wt_sb[k][:], wt_d[k])
                nc.sync.dma_start(u_sb[k][:], u_d[k])

            a_tiles = {}

            def emit_mm_drain(t):
                # ---- matmul K=512, output 4096 f32 in 4 PSUM tiles
                a = st_pool.tile([128, 2 * DIM], BF16, tag="a")
                a_tiles[t] = a
                for pi in range(NPS):
                    pmm = pmm_pool.tile([128, PSW], F32)
                    # k outer: consecutive matmuls share stationary weights
                    for k in range(KT):
                        for mi in range(NMM):
                            cl = pi * PSW + mi * 512
                            nc.tensor.matmul(
                                pmm[:, mi * 512:(mi + 1) * 512],
                                wt_sb[k][:, t * 128:(t + 1) * 128],
                                u_sb[k][:, cl:cl + 512],
                                start=(k == 0), stop=(k == KT - 1))
                    # drain (f32 -> bf16) on ScalarE (GpSimd can't read PSUM)
                    nc.scalar.copy(a[:, pi * PSW:(pi + 1) * PSW], pmm[:])

            def emit_gates(t):
                col = t * N
                c1 = cos_sb[:, col + 1:col + 2]
                s1p = sin_sb[:, col + 1:col + 2]
                s1n = nsin_sb[:, col + 1:col + 2]
                c0 = cos_sb[:, col + 0:col + 1]
                s0p = sin_sb[:, col + 0:col + 1]
                s0n = nsin_sb[:, col + 0:col + 1]
                a = a_tiles.pop(t)
                av = a[:].rearrange("p (c r) -> p c r", r=2)
                # ---- gate q=1: quarters of 512 complex, partner = reversal
                # within the other quarter of the same half, sign -++-
                t1 = st_pool.tile([128, 2 * DIM], BF16, tag="t1")
                nc.vector.tensor_scalar_mul(t1[:], a[:], c1)
                g1 = st_pool.tile([128, 2 * DIM], BF16, tag="g1")
                gv = g1[:].rearrange("p (c r) -> p c r", r=2)
                Q = 512
                quarter_src = [(1, s1n), (0, s1p), (3, s1p), (2, s1n)]
                for qi, (pq, ssel) in enumerate(quarter_src):
                    src = av[:, pq * Q:(pq + 1) * Q, :][:, ::-1, :]
                    nc.vector.tensor_scalar_mul(
                        gv[:, qi * Q:(qi + 1) * Q, :], src, ssel)
                nc.vector.tensor_tensor(g1[:], g1[:], t1[:], ADD)

                # ---- gate q=0: halves of 1024 complex, partner = global
                # reversal, sign -+; t2 on ScalarE (DVE for the last tile:
                # shortens the pipeline tail), fused mul-add on DVE
                t2 = st_pool.tile([128, 2 * DIM], BF16, tag="t2")
                if t == NTILES - 1:
                    nc.vector.tensor_scalar_mul(t2[:], g1[:], c0)
                else:
                    nc.scalar.mul(t2[:], g1[:], c0)
                ot = out_pool.tile([128, 2 * DIM], F32, tag="o")
                ov = ot[:].rearrange("p (c r) -> p c r", r=2)
                t2v = t2[:].rearrange("p (c r) -> p c r", r=2)
                H = 1024
                nc.vector.scalar_tensor_tensor(
                    ov[:, 0:H, :],
                    gv[:, H:2 * H, :][:, ::-1, :], s0n, t2v[:, 0:H, :],
                    MUL, ADD)
                nc.sync.dma_start(out_d[t * 128:(t + 1) * 128, 0:2 * H],
                                  ot[:, 0:2 * H])
                nc.vector.scalar_tensor_tensor(
                    ov[:, H:2 * H, :],
                    gv[:, 0:H, :][:, ::-1, :], s0p, t2v[:, H:2 * H, :],
                    MUL, ADD)
                nc.sync.dma_start(out_d[t * 128:(t + 1) * 128, 2 * H:],
                                  ot[:, 2 * H:])

            # software pipeline: gates for tile t-1 emitted after tile t's
            # matmuls+drains so no engine stalls at the head of its queue
            for t in range(NTILES):
                emit_mm_drain(t)
                if t >= 1:
                    emit_gates(t - 1)
            emit_gates(NTILES - 1)
    nc.finalize()
    return nc

# ----------------------------------------------------------------- driver

_CACHE = {}


def make_in_maps(X, params):
    X = np.ascontiguousarray(np.asarray(X, dtype=np.float32))
    params = np.asarray(params, dtype=np.float32)
    U = build_u_matrix(params)
    u_bf = np.ascontiguousarray(
        U.reshape(KT, 128, 2 * DIM).astype(ml_dtypes.bfloat16))
    WT = build_wt(X)
    in_maps = []
    for c in range(NCORES):
        in_maps.append({
            "x": X[c * BSH:(c + 1) * BSH],
            "wt": np.ascontiguousarray(
                WT[:, :, c * BSH:(c + 1) * BSH].astype(ml_dtypes.bfloat16)),
            "u": u_bf,
        })
    return in_maps


def kernel(X, params):
    if "nc" not in _CACHE:
        _CACHE["nc"] = build_kernel()
    nc = _CACHE["nc"]
    in_maps = make_in_maps(X, params)
    res = run_bass_kernel_spmd(nc, in_maps, list(range(NCORES)))
    out = np.concatenate([res.results[c]["out"] for c in range(NCORES)], axis=0)
    return out.reshape(BATCH, DIM, 2)
